# revision 16
# baseline (speedup 1.0000x reference)
"""DMoLE Linear (base W + masked multi-expert LoRA) on 8 Trainium2 NeuronCores.

Strategy (per sharding hint): data-parallel shard x over the 8192 flattened
tokens (1024 tokens/core); replicate W, b, and the tiny rank-16 LoRA tensors.
Each core computes a disjoint token-slice of the output, so no collectives.

Math per core (T=1024 tokens, D=2048, O=2048, E*R=128):
    y = x @ W^T + b + (x @ A_all^T * mask) @ B_all^T          (SCALING = 1.0)
The per-expert sum collapses: concatenating the E experts along the rank axis
gives A_all [E*R, D], B_all [O, E*R]; the LoRA delta is one extra K=128 step
accumulated into the same PSUM group as the 16 K=128 steps of the base matmul.
The expert mask is folded into A_all on the host (input marshaling).

All matmul operands are bf16 (1 cycle/row on the PE, like f32r, but half the
HBM traffic and a 2x faster FWL weight load; measured end-to-end rel err vs
the fp32 reference is ~3.4e-3, well inside the 2e-2 gate).  x is transposed
on the host to d-major (pure input marshaling), which removes the on-chip PE
identity transposes entirely (the PE runs nothing but the 576 productive
matmuls).  The output is computed o-major ([O, T] per core, un-transposed on
the host): with o on the PSUM partition axis the bias add is a per-partition
scalar op, so eviction is a single Identity-activation (Scalar) or
tensor_scalar_add (DVE) that also casts to the bf16 output tile.

Schedule: the sync DMA queue interleaves one x k-tile with the matching
W o-chunk-0 k-tile so the PE's first accumulation groups start ~1.5 us in;
phase 1 advances three o-blocks (6 PSUM banks) k-step-by-k-step behind the
DMA stream, the two z=x@A^T groups (2 remaining banks) run right after x
lands, and the remaining 13 o-blocks stream with W fully resident.  PSUM
eviction alternates Scalar/DVE per 512-token half; output DMAs ride the
scalar queue.  Predicted ~132-137 us/core HW exec vs 184.8 us for the f32r
on-chip-transpose version.
"""

import os
import numpy as np

B, S, D, O, E, R = 4, 2048, 2048, 2048, 8, 16
ER = E * R                      # 128
NCORES = 8
TOK = B * S                     # 8192
T = TOK // NCORES               # 1024 tokens per core
P = 128
KD = D // P                     # 16 k-tiles
OB = O // P                     # 16 o-blocks of 128
NTG = T // 512                  # 2 512-token groups

_CACHE = {}

# Set by kernel() when KERNEL_TRACE=1: (exec_time_ns, mean_exec_time_ns, tmpdir)
LAST_TIMING = None


def _build():
    from contextlib import ExitStack
    import concourse.tile as tile
    from concourse import bacc, mybir

    F32 = mybir.dt.float32
    BF16 = mybir.dt.bfloat16

    nc = bacc.Bacc("TRN2", target_bir_lowering=False, debug=False)

    xt_d = nc.dram_tensor("xt", [D, T], BF16, kind="ExternalInput").ap()    # x^T
    wt_d = nc.dram_tensor("wt", [D, O], BF16, kind="ExternalInput").ap()    # W^T
    at_d = nc.dram_tensor("at", [P, KD * ER], BF16, kind="ExternalInput").ap()  # (mask*A)^T, sbuf layout
    bt_d = nc.dram_tensor("bt", [ER, O], BF16, kind="ExternalInput").ap()   # B^T
    bias_d = nc.dram_tensor("bias", [P, OB], F32, kind="ExternalInput").ap()
    yt_d = nc.dram_tensor("yt", [O, T], BF16, kind="ExternalOutput").ap()   # y^T

    with tile.TileContext(nc) as tc, ExitStack() as ctx:
        const = ctx.enter_context(tc.tile_pool(name="const", bufs=1))
        big = ctx.enter_context(tc.tile_pool(name="big", bufs=1))
        outp = ctx.enter_context(tc.tile_pool(name="outp", bufs=4))
        ps_y = ctx.enter_context(tc.tile_pool(name="ps_y", bufs=6, space="PSUM"))
        ps_z = ctx.enter_context(tc.tile_pool(name="ps_z", bufs=2, space="PSUM"))

        # x_sb[:, k*T + t] = x[t, k*128 + p];  w_sb[:, k*O + o] = W^T[k*128+p, o]
        x_sb = big.tile([P, KD * T], BF16)
        w_sb = big.tile([P, KD * O], BF16)
        zT = big.tile([ER, T], BF16)

        # The scalar queue's program head runs ~1 us before sync's, so the
        # very first (W, x) k-pair rides scalar — the PE's first real matmul
        # is gated on exactly these two tiles.  at (needed by the z matmuls
        # from the first k-round) follows; it arrives already in SBUF layout
        # [d-in-tile, (k, er)] (host marshaling) so it is one plain
        # 128-descriptor DMA.
        nc.scalar.dma_start(
            out=w_sb[:, 0:512],
            in_=wt_d[0:P, 0:512],
        )
        nc.scalar.dma_start(
            out=x_sb[:, 0:T],
            in_=xt_d[0:P, :],
        )
        at_sb = const.tile([P, KD * ER], BF16)
        nc.scalar.dma_start(out=at_sb[:], in_=at_d[:])
        bt_sb = const.tile([ER, O], BF16)
        bias_sb = const.tile([P, OB], F32)      # column j = b[j*128:(j+1)*128]

        # Input stream on sync, in phase-1 demand order: one W o-chunk-0
        # k-tile paired with the matching x k-tile per round (the pair is
        # exactly one phase-1 k-step's data), then W o-chunks 1-3 staged in
        # PE consumption order.  Everything on ONE queue: x and W combined
        # oversubscribe HBM if streamed concurrently from two queues, which
        # starves the o-chunk the PE needs next.
        for k in range(1, KD):
            nc.sync.dma_start(
                out=w_sb[:, k * O:k * O + 512],
                in_=wt_d[k * P:(k + 1) * P, 0:512],
            )
            nc.sync.dma_start(
                out=x_sb[:, k * T:(k + 1) * T],
                in_=xt_d[k * P:(k + 1) * P, :],
            )
        # bt/bias next (needed by the first deltas/evicts at ~47 us, and off
        # the head of the stream so they don't slow the pair cadence), then
        # W o-chunks 1-3 staged in PE consumption order.
        nc.sync.dma_start(out=bt_sb[:], in_=bt_d[:])
        nc.sync.dma_start(out=bias_sb[:], in_=bias_d[:])
        for oc in range(1, 4):
            for k in range(KD):
                nc.sync.dma_start(
                    out=w_sb[:, k * O + oc * 512:k * O + (oc + 1) * 512],
                    in_=wt_d[k * P:(k + 1) * P, oc * 512:(oc + 1) * 512],
                )

        def base_mm(yp, ob, tg, k):
            nc.tensor.matmul(
                yp[:],
                w_sb[:, k * O + ob * P:k * O + (ob + 1) * P],
                x_sb[:, k * T + tg * 512:k * T + (tg + 1) * 512],
                start=(k == 0),
                stop=False,
            )

        def delta_mm(yp, ob, tg):
            nc.tensor.matmul(
                yp[:],
                bt_sb[:, ob * P:(ob + 1) * P],
                zT[:, tg * 512:(tg + 1) * 512],
                start=False,
                stop=True,
            )

        def evict(ot, yp, ob, tg):
            # o is the partition axis, so the bias add is a per-partition
            # scalar; alternate engines so neither paces the PE.
            dst = ot[:, tg * 512:(tg + 1) * 512]
            bcol = bias_sb[:, ob:ob + 1]
            if tg == 0:
                nc.vector.tensor_scalar_add(dst, yp[:], bcol)
            else:
                # the store rides the scalar queue, so the last eviction of
                # each o-block chains into its store without an engine hop
                nc.scalar.add(dst, yp[:], bcol)

        def store_half(ot, ob, tg):
            nc.scalar.dma_start(
                out=yt_d[ob * P:(ob + 1) * P, tg * 512:(tg + 1) * 512],
                in_=ot[:, tg * 512:(tg + 1) * 512],
            )

        def store_full(ot, ob):
            nc.scalar.dma_start(out=yt_d[ob * P:(ob + 1) * P, :], in_=ot[:])

        # PE warm-up: the HAM clock gate runs the PE at 1.2 GHz until it has
        # seen ~3.4 us of sustained busy.  The first real matmul can't start
        # before ~12 us (framework preamble + first DMAs + semaphore wakes),
        # but the PE queue itself is live from ~6 us — so burn the dead zone
        # on matmuls over a memset tile (no DMA dependency) and the real
        # stream starts at the warm 2.4 GHz rate.
        warm_in = const.tile([P, 512], BF16)
        nc.gpsimd.memset(warm_in[:], 0)
        warm_ps = ps_z.tile([ER, 512], F32, tag="zp", name="warm_ps")
        for _ in range(6):
            nc.tensor.matmul(
                warm_ps[:], warm_in[:, 0:P], warm_in[:], start=True, stop=True
            )

        # Phase 1: three o-blocks on 6 ps_y banks advance k-step-by-k-step
        # behind the (W oc0, x) pair stream, with the two z accumulation
        # groups (2 ps_z banks) folded into the SAME k-loop as a fourth
        # o-block: the input pair cadence is descriptor-rate-bound at
        # ~1.6 us/k while 6 base matmuls only cost 1.3 us, so the z matmuls
        # convert what would be a per-k stall into useful work.
        NP1 = 3
        yps = {}
        for ob in range(NP1):
            for tg in range(NTG):
                yps[(ob, tg)] = ps_y.tile([P, 512], F32, tag="yp", name="yp")
        zps = {}
        for tg in range(NTG):
            zps[tg] = ps_z.tile([ER, 512], F32, tag="zp", name="zp")
        for k in range(KD):
            for tg in range(NTG):
                for ob in range(NP1):
                    base_mm(yps[(ob, tg)], ob, tg, k)
                nc.tensor.matmul(
                    zps[tg][:],
                    at_sb[:, k * ER:(k + 1) * ER],
                    x_sb[:, k * T + tg * 512:k * T + (tg + 1) * 512],
                    start=(k == 0),
                    stop=(k == KD - 1),
                )
        # cast z to bf16 while evicting PSUM
        for tg in range(NTG):
            nc.vector.tensor_copy(zT[:, tg * 512:(tg + 1) * 512], zps[tg][:])

        # o-block 3 runs on the two ps_z banks as they free (its tg0 group
        # waits only for the zT tg0 eviction): 32 dependency-free matmuls
        # that cover the zT eviction + semaphore drain before the deltas.
        yp3 = {}
        for tg in range(NTG):
            yp3[tg] = ps_z.tile([ER, 512], F32, tag="zp", name="yp3")
        for k in range(KD):
            for tg in range(NTG):
                base_mm(yp3[tg], NP1, tg, k)

        # Close phase 1 + o-block 3: deltas, evictions, full-width stores.
        ots = {ob: outp.tile([P, T], BF16, tag="ot", name="ot")
               for ob in range(NP1 + 1)}
        for tg in range(NTG):
            for ob in range(NP1):
                delta_mm(yps[(ob, tg)], ob, tg)
            delta_mm(yp3[tg], NP1, tg)
        for tg in range(NTG):
            for ob in range(NP1):
                evict(ots[ob], yps[(ob, tg)], ob, tg)
            evict(ots[NP1], yp3[tg], NP1, tg)
        for ob in range(NP1 + 1):
            store_full(ots[ob], ob)

        # Phase 2: remaining 12 o-blocks; W is resident (or arrives well
        # ahead of the PE).  Two groups per o-block share each stationary
        # W tile across the two 512-token moving halves.
        for ob in range(NP1 + 1, OB):
            ypA = ps_y.tile([P, 512], F32, tag="yp")
            ypB = ps_y.tile([P, 512], F32, tag="yp")
            yp2 = {0: ypA, 1: ypB}
            for k in range(KD):
                for tg in range(NTG):
                    base_mm(yp2[tg], ob, tg, k)
            ot = outp.tile([P, T], BF16, tag="ot")
            if ob < OB - 1:
                for tg in range(NTG):
                    delta_mm(yp2[tg], ob, tg)
                    evict(ot, yp2[tg], ob, tg)
                store_full(ot, ob)
            else:
                # last o-block: per-half stores so the final evict chains
                # straight into a small store instead of one late 256 KB DMA
                for tg in range(NTG):
                    delta_mm(yp2[tg], ob, tg)
                    evict(ot, yp2[tg], ob, tg)
                    store_half(ot, ob, tg)

    nc.compile()
    return nc


def _get_nc():
    if "nc" not in _CACHE:
        _CACHE["nc"] = _build()
    return _CACHE["nc"]


def kernel(x, W, b, lora_A, lora_B, expert_mask):
    global LAST_TIMING
    import ml_dtypes
    from concourse.bass_utils import run_bass_kernel_spmd

    BF = ml_dtypes.bfloat16
    nc = _get_nc()

    x = np.asarray(x, dtype=np.float32)
    W = np.asarray(W, dtype=np.float32)
    b = np.asarray(b, dtype=np.float32)
    lora_A = np.asarray(lora_A, dtype=np.float32)
    lora_B = np.asarray(lora_B, dtype=np.float32)
    maskf = np.asarray(expert_mask).astype(np.float32)

    xf = x.reshape(TOK, D)
    xbf = xf.astype(BF)
    wt = np.ascontiguousarray(W.astype(BF).T)                     # [D, O]
    mA = lora_A * maskf[:, None, None]                            # fold mask
    atDxER = np.transpose(mA, (2, 0, 1)).reshape(D, ER)           # [D, ER]
    at = np.ascontiguousarray(                                    # [P, KD*ER]
        atDxER.reshape(KD, P, ER).transpose(1, 0, 2).reshape(P, KD * ER)
    ).astype(BF)
    bt = np.ascontiguousarray(
        np.transpose(lora_B, (0, 2, 1)).reshape(ER, O).astype(BF))  # [ER, O]
    bias = np.ascontiguousarray(b.reshape(OB, P).T)               # [P, OB] f32
    shared = {"wt": wt, "at": at, "bt": bt, "bias": bias}
    in_maps = [
        {"xt": np.ascontiguousarray(xbf[i * T:(i + 1) * T].T), **shared}
        for i in range(NCORES)
    ]

    trace = os.environ.get("KERNEL_TRACE", "0") == "1"
    kw = {}
    if trace:
        import sys
        import types
        import tempfile

        if "antenv.axon_hooks" not in sys.modules:
            import trn_agent_boot.trn_boot as tb

            hook = tb._ntff_profile_via_ctypes("/opt/axon/libaxon_pjrt.so")
            mod = types.ModuleType("antenv.axon_hooks")
            mod.get_axon_ntff_profile_hook = lambda: hook
            sys.modules["antenv.axon_hooks"] = mod
        kw = {"trace": True, "tmpdir": tempfile.mkdtemp(prefix="dmole_trace_")}

    def spot_check(y2d):
        # Cheap host-side guard against rare transient device flakes: verify
        # a few output rows (one per pair of cores) against a CPU compute.
        # bf16 rounding alone contributes ~3e-3, so gate at 2e-2.
        for t in range(T // 2, TOK, 2 * T):
            row = xf[t]
            ref = row @ W.T + b
            z = np.einsum("erd,d->er", mA, row)
            ref = ref + np.einsum("eor,er->o", lora_B, z)
            scale = max(np.abs(ref).max(), 1e-6)
            if np.abs(y2d[t] - ref).max() / scale > 2e-2:
                return False
        return True

    res = None
    y = None
    for attempt in range(3):
        try:
            res = run_bass_kernel_spmd(nc, in_maps, list(range(NCORES)), **kw)
        except Exception:
            # A transiently wedged NeuronCore (NRT_EXEC_UNIT_*) is usually
            # fine on the next load/execute.
            if attempt == 2:
                raise
            continue
        y = np.empty((TOK, O), dtype=np.float32)
        for i in range(NCORES):
            y[i * T:(i + 1) * T] = res.results[i]["yt"].T.astype(np.float32)
        if spot_check(y):
            break
    if trace:
        LAST_TIMING = (res.exec_time_ns, res.mean_exec_time_ns, kw.get("tmpdir"))

    return np.ascontiguousarray(y.reshape(B, S, O), dtype=np.float32)


# revision 17
# speedup vs baseline: 1.0198x; 1.0198x over previous
"""DMoLE Linear (base W + masked multi-expert LoRA) on 8 Trainium2 NeuronCores.

Strategy (per sharding hint): data-parallel shard x over the 8192 flattened
tokens (1024 tokens/core); replicate W, b, and the tiny rank-16 LoRA tensors.
Each core computes a disjoint token-slice of the output, so no collectives.

Math per core (T=1024 tokens, D=2048, O=2048, E*R=128):
    y = x @ W^T + b + (x @ A_all^T * mask) @ B_all^T          (SCALING = 1.0)
The per-expert sum collapses: concatenating the E experts along the rank axis
gives A_all [E*R, D], B_all [O, E*R]; the LoRA delta is one extra K=128 step
accumulated into the same PSUM group as the 16 K=128 steps of the base matmul.
The expert mask is folded into A_all on the host (input marshaling).

All matmul operands are bf16 (1 cycle/row on the PE, like f32r, but half the
HBM traffic and a 2x faster FWL weight load; measured end-to-end rel err vs
the fp32 reference is ~3.4e-3, well inside the 2e-2 gate).  x is transposed
on the host to d-major (pure input marshaling), which removes the on-chip PE
identity transposes entirely (the PE runs nothing but the 576 productive
matmuls).  The output is computed o-major ([O, T] per core, un-transposed on
the host): with o on the PSUM partition axis the bias add is a per-partition
scalar op, so eviction is a single Identity-activation (Scalar) or
tensor_scalar_add (DVE) that also casts to the bf16 output tile.

Schedule: the sync DMA queue interleaves one x k-tile with the matching
W o-chunk-0 k-tile so the PE's first accumulation groups start ~1.5 us in;
phase 1 advances three o-blocks (6 PSUM banks) k-step-by-k-step behind the
DMA stream, the two z=x@A^T groups (2 remaining banks) run right after x
lands, and the remaining 13 o-blocks stream with W fully resident.  PSUM
eviction alternates Scalar/DVE per 512-token half; output DMAs ride the
scalar queue.  Predicted ~132-137 us/core HW exec vs 184.8 us for the f32r
on-chip-transpose version.
"""

import os
import numpy as np

B, S, D, O, E, R = 4, 2048, 2048, 2048, 8, 16
ER = E * R                      # 128
NCORES = 8
TOK = B * S                     # 8192
T = TOK // NCORES               # 1024 tokens per core
P = 128
KD = D // P                     # 16 k-tiles
OB = O // P                     # 16 o-blocks of 128
NTG = T // 512                  # 2 512-token groups

_CACHE = {}

# Set by kernel() when KERNEL_TRACE=1: (exec_time_ns, mean_exec_time_ns, tmpdir)
LAST_TIMING = None


def _build():
    from contextlib import ExitStack
    import concourse.tile as tile
    from concourse import bacc, mybir

    F32 = mybir.dt.float32
    BF16 = mybir.dt.bfloat16

    nc = bacc.Bacc("TRN2", target_bir_lowering=False, debug=False)

    xt_d = nc.dram_tensor("xt", [D, T], BF16, kind="ExternalInput").ap()    # x^T
    wt_d = nc.dram_tensor("wt", [D, O], BF16, kind="ExternalInput").ap()    # W^T
    at_d = nc.dram_tensor("at", [P, KD * ER], BF16, kind="ExternalInput").ap()  # (mask*A)^T, sbuf layout
    bt_d = nc.dram_tensor("bt", [ER, O], BF16, kind="ExternalInput").ap()   # B^T
    bias_d = nc.dram_tensor("bias", [P, OB], F32, kind="ExternalInput").ap()
    yt_d = nc.dram_tensor("yt", [O, T], BF16, kind="ExternalOutput").ap()   # y^T

    with tile.TileContext(nc) as tc, ExitStack() as ctx:
        const = ctx.enter_context(tc.tile_pool(name="const", bufs=1))
        big = ctx.enter_context(tc.tile_pool(name="big", bufs=1))
        outp = ctx.enter_context(tc.tile_pool(name="outp", bufs=4))
        ps_y = ctx.enter_context(tc.tile_pool(name="ps_y", bufs=6, space="PSUM"))
        ps_z = ctx.enter_context(tc.tile_pool(name="ps_z", bufs=2, space="PSUM"))

        # x_sb[:, k*T + t] = x[t, k*128 + p];  w_sb[:, k*O + o] = W^T[k*128+p, o]
        x_sb = big.tile([P, KD * T], BF16)
        w_sb = big.tile([P, KD * O], BF16)
        zT = big.tile([ER, T], BF16)

        # at rides the scalar queue (needed by the z matmuls from the first
        # k-round); it arrives already in SBUF layout [d-in-tile, (k, er)]
        # (host marshaling) so this is one plain 128-descriptor DMA.
        at_sb = const.tile([P, KD * ER], BF16)
        nc.scalar.dma_start(out=at_sb[:], in_=at_d[:])
        bt_sb = const.tile([ER, O], BF16)
        bias_sb = const.tile([P, OB], F32)      # column j = b[j*128:(j+1)*128]

        # Input stream on sync, in phase-1 demand order: one W o-chunk-0
        # k-tile paired with the matching x k-tile per round (the pair is
        # exactly one phase-1 k-step's data), then W o-chunks 1-3 staged in
        # PE consumption order.  Everything on ONE queue: x and W combined
        # oversubscribe HBM if streamed concurrently from two queues, which
        # starves the o-chunk the PE needs next.
        for k in range(KD):
            nc.sync.dma_start(
                out=w_sb[:, k * O:k * O + 512],
                in_=wt_d[k * P:(k + 1) * P, 0:512],
            )
            nc.sync.dma_start(
                out=x_sb[:, k * T:(k + 1) * T],
                in_=xt_d[k * P:(k + 1) * P, :],
            )
        # bt/bias next (needed by the first deltas/evicts at ~47 us, and off
        # the head of the stream so they don't slow the pair cadence), then
        # W o-chunks 1-3 staged in PE consumption order.
        nc.sync.dma_start(out=bt_sb[:], in_=bt_d[:])
        nc.sync.dma_start(out=bias_sb[:], in_=bias_d[:])
        for oc in range(1, 4):
            for k in range(KD):
                nc.sync.dma_start(
                    out=w_sb[:, k * O + oc * 512:k * O + (oc + 1) * 512],
                    in_=wt_d[k * P:(k + 1) * P, oc * 512:(oc + 1) * 512],
                )

        def base_mm(yp, ob, tg, k):
            nc.tensor.matmul(
                yp[:],
                w_sb[:, k * O + ob * P:k * O + (ob + 1) * P],
                x_sb[:, k * T + tg * 512:k * T + (tg + 1) * 512],
                start=(k == 0),
                stop=False,
            )

        def delta_mm(yp, ob, tg):
            nc.tensor.matmul(
                yp[:],
                bt_sb[:, ob * P:(ob + 1) * P],
                zT[:, tg * 512:(tg + 1) * 512],
                start=False,
                stop=True,
            )

        def evict(ot, yp, ob, tg):
            # o is the partition axis, so the bias add is a per-partition
            # scalar; alternate engines so neither paces the PE.
            dst = ot[:, tg * 512:(tg + 1) * 512]
            bcol = bias_sb[:, ob:ob + 1]
            if tg == 0:
                nc.vector.tensor_scalar_add(dst, yp[:], bcol)
            else:
                # the store rides the scalar queue, so the last eviction of
                # each o-block chains into its store without an engine hop
                nc.scalar.add(dst, yp[:], bcol)

        def store_half(ot, ob, tg):
            nc.scalar.dma_start(
                out=yt_d[ob * P:(ob + 1) * P, tg * 512:(tg + 1) * 512],
                in_=ot[:, tg * 512:(tg + 1) * 512],
            )

        def store_full(ot, ob):
            nc.scalar.dma_start(out=yt_d[ob * P:(ob + 1) * P, :], in_=ot[:])

        # PE warm-up: the HAM clock gate runs the PE at 1.2 GHz until it has
        # seen ~3.4 us of sustained busy.  The first real matmul can't start
        # before ~12 us (framework preamble + first DMAs + semaphore wakes),
        # but the PE queue itself is live from ~6 us — so burn the dead zone
        # on matmuls over a memset tile (no DMA dependency) and the real
        # stream starts at the warm 2.4 GHz rate.
        warm_in = const.tile([P, 512], BF16)
        nc.gpsimd.memset(warm_in[:], 0)
        warm_ps = ps_z.tile([ER, 512], F32, tag="zp", name="warm_ps")
        for _ in range(10):
            nc.tensor.matmul(
                warm_ps[:], warm_in[:, 0:P], warm_in[:], start=True, stop=True
            )

        # Phase 1: three o-blocks on 6 ps_y banks advance k-step-by-k-step
        # behind the (W oc0, x) pair stream, with the two z accumulation
        # groups (2 ps_z banks) folded into the SAME k-loop as a fourth
        # o-block: the input pair cadence is descriptor-rate-bound at
        # ~1.6 us/k while 6 base matmuls only cost 1.3 us, so the z matmuls
        # convert what would be a per-k stall into useful work.
        NP1 = 3
        yps = {}
        for ob in range(NP1):
            for tg in range(NTG):
                yps[(ob, tg)] = ps_y.tile([P, 512], F32, tag="yp", name="yp")
        zps = {}
        for tg in range(NTG):
            zps[tg] = ps_z.tile([ER, 512], F32, tag="zp", name="zp")
        for k in range(KD):
            for tg in range(NTG):
                for ob in range(NP1):
                    base_mm(yps[(ob, tg)], ob, tg, k)
                nc.tensor.matmul(
                    zps[tg][:],
                    at_sb[:, k * ER:(k + 1) * ER],
                    x_sb[:, k * T + tg * 512:k * T + (tg + 1) * 512],
                    start=(k == 0),
                    stop=(k == KD - 1),
                )
        # cast z to bf16 while evicting PSUM
        for tg in range(NTG):
            nc.vector.tensor_copy(zT[:, tg * 512:(tg + 1) * 512], zps[tg][:])

        # o-block 3 runs on the two ps_z banks as they free (its tg0 group
        # waits only for the zT tg0 eviction): 32 dependency-free matmuls
        # that cover the zT eviction + semaphore drain before the deltas.
        yp3 = {}
        for tg in range(NTG):
            yp3[tg] = ps_z.tile([ER, 512], F32, tag="zp", name="yp3")
        for k in range(KD):
            for tg in range(NTG):
                base_mm(yp3[tg], NP1, tg, k)

        # Close phase 1 + o-block 3: deltas, evictions, full-width stores.
        ots = {ob: outp.tile([P, T], BF16, tag="ot", name="ot")
               for ob in range(NP1 + 1)}
        for tg in range(NTG):
            for ob in range(NP1):
                delta_mm(yps[(ob, tg)], ob, tg)
            delta_mm(yp3[tg], NP1, tg)
        for tg in range(NTG):
            for ob in range(NP1):
                evict(ots[ob], yps[(ob, tg)], ob, tg)
            evict(ots[NP1], yp3[tg], NP1, tg)
        for ob in range(NP1 + 1):
            store_full(ots[ob], ob)

        # Phase 2: remaining 12 o-blocks; W is resident (or arrives well
        # ahead of the PE).  Two groups per o-block share each stationary
        # W tile across the two 512-token moving halves.
        for ob in range(NP1 + 1, OB):
            ypA = ps_y.tile([P, 512], F32, tag="yp")
            ypB = ps_y.tile([P, 512], F32, tag="yp")
            yp2 = {0: ypA, 1: ypB}
            for k in range(KD):
                for tg in range(NTG):
                    base_mm(yp2[tg], ob, tg, k)
            ot = outp.tile([P, T], BF16, tag="ot")
            if ob < OB - 1:
                for tg in range(NTG):
                    delta_mm(yp2[tg], ob, tg)
                    evict(ot, yp2[tg], ob, tg)
                store_full(ot, ob)
            else:
                # last o-block: per-half stores so the final evict chains
                # straight into a small store instead of one late 256 KB DMA
                for tg in range(NTG):
                    delta_mm(yp2[tg], ob, tg)
                    evict(ot, yp2[tg], ob, tg)
                    store_half(ot, ob, tg)

    nc.compile()
    return nc


def _get_nc():
    if "nc" not in _CACHE:
        _CACHE["nc"] = _build()
    return _CACHE["nc"]


def kernel(x, W, b, lora_A, lora_B, expert_mask):
    global LAST_TIMING
    import ml_dtypes
    from concourse.bass_utils import run_bass_kernel_spmd

    BF = ml_dtypes.bfloat16
    nc = _get_nc()

    x = np.asarray(x, dtype=np.float32)
    W = np.asarray(W, dtype=np.float32)
    b = np.asarray(b, dtype=np.float32)
    lora_A = np.asarray(lora_A, dtype=np.float32)
    lora_B = np.asarray(lora_B, dtype=np.float32)
    maskf = np.asarray(expert_mask).astype(np.float32)

    xf = x.reshape(TOK, D)
    xbf = xf.astype(BF)
    wt = np.ascontiguousarray(W.astype(BF).T)                     # [D, O]
    mA = lora_A * maskf[:, None, None]                            # fold mask
    atDxER = np.transpose(mA, (2, 0, 1)).reshape(D, ER)           # [D, ER]
    at = np.ascontiguousarray(                                    # [P, KD*ER]
        atDxER.reshape(KD, P, ER).transpose(1, 0, 2).reshape(P, KD * ER)
    ).astype(BF)
    bt = np.ascontiguousarray(
        np.transpose(lora_B, (0, 2, 1)).reshape(ER, O).astype(BF))  # [ER, O]
    bias = np.ascontiguousarray(b.reshape(OB, P).T)               # [P, OB] f32
    shared = {"wt": wt, "at": at, "bt": bt, "bias": bias}
    in_maps = [
        {"xt": np.ascontiguousarray(xbf[i * T:(i + 1) * T].T), **shared}
        for i in range(NCORES)
    ]

    trace = os.environ.get("KERNEL_TRACE", "0") == "1"
    kw = {}
    if trace:
        import sys
        import types
        import tempfile

        if "antenv.axon_hooks" not in sys.modules:
            import trn_agent_boot.trn_boot as tb

            hook = tb._ntff_profile_via_ctypes("/opt/axon/libaxon_pjrt.so")
            mod = types.ModuleType("antenv.axon_hooks")
            mod.get_axon_ntff_profile_hook = lambda: hook
            sys.modules["antenv.axon_hooks"] = mod
        kw = {"trace": True, "tmpdir": tempfile.mkdtemp(prefix="dmole_trace_")}

    def spot_check(y2d):
        # Cheap host-side guard against rare transient device flakes: verify
        # a few output rows (one per pair of cores) against a CPU compute.
        # bf16 rounding alone contributes ~3e-3, so gate at 2e-2.
        for t in range(T // 2, TOK, 2 * T):
            row = xf[t]
            ref = row @ W.T + b
            z = np.einsum("erd,d->er", mA, row)
            ref = ref + np.einsum("eor,er->o", lora_B, z)
            scale = max(np.abs(ref).max(), 1e-6)
            if np.abs(y2d[t] - ref).max() / scale > 2e-2:
                return False
        return True

    res = None
    y = None
    for attempt in range(3):
        try:
            res = run_bass_kernel_spmd(nc, in_maps, list(range(NCORES)), **kw)
        except Exception:
            # A transiently wedged NeuronCore (NRT_EXEC_UNIT_*) is usually
            # fine on the next load/execute.
            if attempt == 2:
                raise
            continue
        y = np.empty((TOK, O), dtype=np.float32)
        for i in range(NCORES):
            y[i * T:(i + 1) * T] = res.results[i]["yt"].T.astype(np.float32)
        if spot_check(y):
            break
    if trace:
        LAST_TIMING = (res.exec_time_ns, res.mean_exec_time_ns, kw.get("tmpdir"))

    return np.ascontiguousarray(y.reshape(B, S, O), dtype=np.float32)
